# revision 25
# baseline (speedup 1.0000x reference)
"""Trainium2 Bass kernel for nn_EquivariantCorrectionHead (v2).

Math: two chained e3nn-style fully-connected tensor products on irreps
128x0e + 8x2e -> Hx0e + Hx2e -> 1x2e, batch B=2048, data-parallel over
8 NeuronCores (256 rows/core = z).

Key structure (per core):
  - tp1 contractions run "transposed": batch z on the FREE axis (256 wide),
    contraction index on partitions, so matmul operands need no on-chip
    transposition and no PSUM->SBUF copy tax for the big pair-product matrix.
  - Pair products s_u*s_v are enumerated by cyclic diagonals d: chunk d holds
    pairs (p, (p+d)%128) on partition p. The host supplies sT and its 64
    rotated copies (rotfat), so each 16-diagonal group is one full-partition
    bf16 tensor_tensor op (2x DVE rate). W000 is host-folded to the same
    diagonal layout (W[u,v]+W[v,u]), halving weight traffic vs dense.
  - W220 path: same diagonal trick over the 8 l=2 channels (x5 components)
    with host-rotated tT; accumulates into the same y0 PSUM.
  - W_PR (W022+W202) path: akT[u,z] = sT[u,z]*t3[z,c,k] products against
    partition-broadcast rows of tT (k=0..2 replicated by the host, k=3,4
    via gpsimd partition_broadcast); y2T accumulates 8 matmuls per (k, half).
  - W222 path: z-major pair products of t + sparse C222 combine (DVE), small
    PE transpose to get tpkT, one matmul per (k, half).
  - tp2 runs z-major: D_i = y2_i @ V222sym and af = y0 @ V02f are matmuls
    with y2T/y0T slices as lhsT; the per-sample contractions
    F_ij = sum_v d_i*y2_j and o01_k = sum_v af*y2_k use scalar_tensor_tensor
    with the fused accum_out reduction (no TensorReduce), split DVE/Pool.
    V222 is symmetrized (C222 is fully symmetric) so only 15 F pairs needed.
  - Final: transpose fmat [z,20] -> [20,z], one matmul with the
    C222/identity combine matrix -> out [z,8] (cols 0..4 valid).
All heavy data is bf16 (fp32 PSUM accumulation); rel err ~6e-3 vs 2e-2 gate.
Small input DMAs are host-packed into a few wide tensors to cut HWDGE
per-DMA overhead; weight/rotation streams are interleaved so the y0 matmul
loop is fed as data arrives.
"""

import sys
import numpy as np
import ml_dtypes

sys.path.insert(0, "/opt/trn_rl_repo")

B = 2048
N_S = 128
H = 256
N_CORES = 8
ZC = B // N_CORES          # 256 batch rows per core
ND = 64                    # rotation diagonals d=1..64
NCH = ND + 1               # 65 pair chunks (d=0..64)

N_L2 = 8
C0 = float(np.sqrt(1.0 / (N_S**2 + N_L2**2)))
C2 = float(np.sqrt(5.0 / (2 * N_S * N_L2 + N_L2**2)))
C2B = float(np.sqrt(5.0 / (3 * H**2)))
INV_S5 = float(1.0 / np.sqrt(5.0))

BF = ml_dtypes.bfloat16

# shared pack128 column offsets (bf16 elements)
OFF_IDENT = 0
OFF_W220 = 128
OFF_V222 = 128 + 3 * 256
OFF_V02F = OFF_V222 + 512
OFF_WPR = OFF_V02F + 512
PACK128_W = OFF_WPR + 2048       # 4224


def _w3j_222():
    Q = np.zeros((5, 3, 3))
    s = 1.0 / np.sqrt(2.0)
    Q[0, 0, 1] = Q[0, 1, 0] = s
    Q[1, 1, 2] = Q[1, 2, 1] = s
    Q[2] = np.diag([-1.0, -1.0, 2.0]) / np.sqrt(6.0)
    Q[3, 0, 2] = Q[3, 2, 0] = s
    Q[4, 0, 0] = s
    Q[4, 1, 1] = -s
    C = np.einsum('aij,bjk,cki->abc', Q, Q, Q)
    return (C / np.linalg.norm(C)).astype(np.float64)


C222 = _w3j_222()  # [i, j, k], fully symmetric
C222_NNZ = [[(i, j, float(C222[i, j, k]))
             for i in range(5) for j in range(5) if C222[i, j, k] != 0.0]
            for k in range(5)]
F_PAIRS = [(i, j) for i in range(5) for j in range(i, 5)]  # 15 rows

# W220 diagonal layout: (d, chunk, quadrant-start)
W220_SLOTS = [(0, 0, 0), (1, 0, 64), (2, 1, 0), (3, 1, 64), (4, 2, 0)]


def host_prep(scalars, kernel_t2s, W000, W220, W022, W202, W222, V022, V202,
              V222):
    """Numpy packing to the device layouts (bf16). Returns
    (shared_weights_dict, per_core_list_of_dicts)."""
    f = np.float32

    # W000 diagonal fold: row (d*128+p) -> pair (p, q=(p+d)%128)
    w000diag = np.zeros((NCH * 128, H), dtype=f)
    p = np.arange(128)
    for d in range(NCH):
        q = (p + d) % 128
        if d == 0:
            blk = W000[p, p, :]
        else:
            blk = W000[p, q, :] + W000[q, p, :]
        if d == 64:
            blk = blk.copy()
            blk[64:] = 0.0
        w000diag[d * 128:(d + 1) * 128] = C0 * blk

    pack128 = np.zeros((128, PACK128_W), dtype=f)
    pack128[:, OFF_IDENT:OFF_IDENT + 128] = np.eye(128, dtype=f)
    # W220 diagonal layout (3 chunks, quadrant starts, i-replicated)
    for d, ch, st in W220_SLOTS:
        c = np.arange(4 if d == 4 else 8)
        cq = (c + d) % 8
        blk = W220[c, cq, :] if d == 0 else W220[c, cq, :] + W220[cq, c, :]
        rep = np.repeat((C0 * INV_S5) * blk, 5, axis=0)
        pack128[st:st + rep.shape[0], OFF_W220 + ch * 256:
                OFF_W220 + (ch + 1) * 256] = rep
    vs = C2B * 0.5 * (V222[:, :, 0] + V222[:, :, 0].T)
    pack128[:, OFF_V222:OFF_V222 + 256] = vs[0:128, :]
    pack128[:, OFF_V222 + 256:OFF_V222 + 512] = vs[128:256, :]
    vf = (C2B * INV_S5) * (V022[:, :, 0] + V202[:, :, 0].T)
    pack128[:, OFF_V02F:OFF_V02F + 256] = vf[0:128, :]
    pack128[:, OFF_V02F + 256:OFF_V02F + 512] = vf[128:256, :]
    wpr = (C2 * INV_S5) * (W022 + W202.transpose(1, 0, 2))   # [u, c, w]
    pack128[:, OFF_WPR:OFF_WPR + 2048] = wpr.reshape(128, 2048)

    w222p = C2 * W222.reshape(64, H)

    comb = np.zeros((20, 8), dtype=f)
    for r, (i, j) in enumerate(F_PAIRS):
        for k in range(5):
            comb[r, k] = C222[i, j, k] * (1.0 if i == j else 2.0)
    for k in range(5):
        comb[15 + k, k] = 1.0

    bf = lambda x: np.ascontiguousarray(x, dtype=BF)
    shared = dict(w000diag=bf(w000diag), pack128=bf(pack128))

    # ---- per-core batch data ----
    t3 = np.concatenate([kernel_t2s[:, :7, :],
                         kernel_t2s.sum(1, keepdims=True)], 1)  # [B, 8, 5]
    percore = []
    for cidx in range(N_CORES):
        rows = slice(cidx * ZC, (cidx + 1) * ZC)
        sT = np.ascontiguousarray(scalars[rows].T, dtype=f)     # [128, 256]
        rot = np.empty((128, ND * 256), dtype=f)
        for d in range(1, ND + 1):
            rot[:, (d - 1) * 256:d * 256] = np.roll(sT, -d, axis=0)
        tc = t3[rows].reshape(ZC, 40)                           # [256, 40]
        tT = np.ascontiguousarray(tc.T, dtype=f)                # [40, 256]
        packA = np.zeros((128, 1888), dtype=f)
        packA[:, 0:256] = sT
        packA[:, 256:296] = tc[0:128]
        packA[:, 296:336] = tc[128:256]
        packA[0:40, 336:592] = tT
        tTr = tT.reshape(8, 5, 256)
        for d in range(1, 5):
            packA[0:40, 336 + d * 256:336 + (d + 1) * 256] = np.roll(
                tTr, -d, axis=0).reshape(40, 256)
        packA[0:20, 1616:1624] = comb
        packA[0:64, 1624:1880] = w222p
        packA[0:5, 1880:1888] = comb[15:20]
        ball5 = np.empty((128, 5 * 2048), dtype=f)
        for k in range(5):
            for c in range(8):
                ball5[:, (k * 8 + c) * 256:(k * 8 + c + 1) * 256] = \
                    tT[c * 5 + k][None, :]
        percore.append(dict(packa=bf(packA), rotfat=bf(rot),
                            ball5=bf(ball5)))
    return shared, percore


def build_nc(repeat=1):
    import concourse.bacc as bacc
    import concourse.tile as tile
    import concourse.mybir as mybir

    f32 = mybir.dt.float32
    bf16 = mybir.dt.bfloat16
    MULT = mybir.AluOpType.mult
    ADD = mybir.AluOpType.add

    nc = bacc.Bacc("TRN2", target_bir_lowering=False, debug=False,
                   num_devices=N_CORES)

    def dram(name, shape, dt=bf16, kind="ExternalInput"):
        return nc.dram_tensor(name, list(shape), dt, kind=kind).ap()

    w000_d = dram("w000diag", [NCH * 128, 256])
    pack128_d = dram("pack128", [128, PACK128_W])
    packa_d = dram("packa", [128, 1888])
    rot_d = dram("rotfat", [128, ND * 256])
    ball_d = dram("ball5", [128, 5 * 2048])
    out_d = dram("out", [ZC, 8], dt=f32, kind="ExternalOutput")

    def stt(eng, out, in0, in1, accum_out=None):
        eng.scalar_tensor_tensor(out, in0, 1.0, in1, op0=MULT, op1=MULT,
                                 accum_out=accum_out)

    def tt_mul(out, in0, in1):
        nc.vector.tensor_tensor(out, in0, in1, op=MULT)

    mm = nc.tensor.matmul

    from contextlib import ExitStack
    with tile.TileContext(nc) as tc, ExitStack() as es:
        es.enter_context(nc.allow_low_precision(
            reason="bf16 storage everywhere; PSUM accumulation is fp32; "
                   "tolerance budget 2e-2 vs expected ~6e-3"))
        if repeat > 1:
            es.enter_context(tc.For_i(0, repeat, 1))
        cpool = es.enter_context(tc.tile_pool(name="consts", bufs=1))
        wstream = es.enter_context(tc.tile_pool(name="wstream", bufs=6))
        big = es.enter_context(tc.tile_pool(name="big", bufs=1))
        scr = es.enter_context(tc.tile_pool(name="scr", bufs=4))
        ptr = es.enter_context(tc.tile_pool(name="ptr", bufs=2, space="PSUM"))
        py0 = es.enter_context(tc.tile_pool(name="py0", bufs=1, space="PSUM"))
        py2 = es.enter_context(tc.tile_pool(name="py2", bufs=2, space="PSUM"))
        p256 = es.enter_context(tc.tile_pool(name="p256", bufs=2,
                                             space="PSUM"))

        dma = nc.sync.dma_start

        # ---- packed small DMAs (pack128 is deferred into the stream) ----
        pk = cpool.tile([128, PACK128_W], bf16, tag="pack128")
        identb = pk[:, OFF_IDENT:OFF_IDENT + 128]
        w220_s = [pk[:, OFF_W220 + ch * 256:OFF_W220 + (ch + 1) * 256]
                  for ch in range(3)]
        v222_s = pk[:, OFF_V222:OFF_V222 + 512]
        v02f_s = pk[:, OFF_V02F:OFF_V02F + 512]
        wpr_s = pk[:, OFF_WPR:OFF_WPR + 2048]
        pka = cpool.tile([128, 1888], bf16, tag="packa")
        dma(out=pka, in_=packa_d)
        st_s = pka[:, 0:256]
        tb_s = pka[:, 256:336]
        p40 = pka[0:40, 336:1616]
        tt_s = p40[:, 0:256]
        comb_s = pka[0:20, 1616:1624]
        combO_s = pka[0:5, 1880:1888]
        w222_s = pka[0:64, 1624:1880]
        # warm the activation table early (one-time 1.3us load)
        warm = scr.tile([1, 8], bf16, tag="warm")
        nc.scalar.copy(out=warm, in_=comb_s[0:1, :])

        # ---- Pool: tt2 pad memsets, broadcasts for k=3,4 ----
        tt2_s = [big.tile([128, 256], bf16, name=f"tt2_{ch}",
                          tag=f"tt2_{ch}") for ch in range(3)]
        for ch in range(3):
            nc.gpsimd.memset(tt2_s[ch], 0.0)
        ball_s = big.tile([128, 5 * 2048], bf16, tag="ball")

        # ---- streams: rotations + w000 interleaved with compute ----
        rot_s = big.tile([128, ND * 256], bf16, tag="rot")
        s2p_s = big.tile([128, NCH * 256], bf16, tag="s2p")
        ak_s = big.tile([128, 40 * 256], bf16, tag="ak")

        def dma_rot(w):
            dma(out=rot_s[:, w * 4096:(w + 1) * 4096],
                in_=rot_d[:, w * 4096:(w + 1) * 4096])

        def dve_s2p(w):
            tt_mul(s2p_s[:, (1 + 16 * w) * 256:(17 + 16 * w) * 256]
                   .rearrange("p (d z) -> p d z", d=16),
                   st_s.unsqueeze(1).to_broadcast([128, 16, 256]),
                   rot_s[:, (16 * w) * 256:(16 * w + 16) * 256]
                   .rearrange("p (d z) -> p d z", d=16))

        W_GROUPS = [(0, 13), (13, 13), (26, 13), (39, 13), (52, 8),
                    (60, 5)]  # small last group -> short post-DMA y0 tail
        wq_s = []

        def dma_w000(g):
            c0, n = W_GROUPS[g]
            wq = wstream.tile([128, n * 256], bf16, name=f"wq{g}", tag="w000")
            wq_s.append(wq)
            dma(out=wq.rearrange("p (c w) -> p c w", c=n),
                in_=w000_d[c0 * 128:(c0 + n) * 128, :]
                .rearrange("(c p) w -> p c w", p=128))

        y0ps = [py0.tile([128, 256], f32, name=f"y0ps{wh}", tag=f"y0_{wh}")
                for wh in range(2)]

        def pe_y0(g):
            c0, n = W_GROUPS[g]
            wq = wq_s[g]
            for jj in range(n):
                c = c0 + jj
                for wh in range(2):
                    mm(y0ps[wh],
                       wq[:, jj * 256 + wh * 128:jj * 256 + wh * 128 + 128],
                       s2p_s[:, c * 256:(c + 1) * 256],
                       start=(c == 0), stop=(c == NCH - 1))

        # kick off the streams (DMA queue order = transfer order)
        def dma_ball(k):
            dma(out=ball_s[:, k * 2048:(k + 1) * 2048],
                in_=ball_d[:, k * 2048:(k + 1) * 2048])

        dma_rot(0)
        dma_w000(0)
        dma_ball(0)
        dma_rot(1)
        dma_w000(1)
        dma(out=pk, in_=pack128_d)
        dma_rot(2)
        dma_ball(1)
        dma_w000(2)
        dma_rot(3)
        dma_ball(2)
        dma_ball(3)
        dma_ball(4)
        dma_w000(3)
        dma_w000(4)
        dma_w000(5)

        # ---- DVE program (order matters: products feed PE as they land) ----
        def tpk_chain(eng, k):
            tpk = tpk_s[k]
            tv = tpk.rearrange("p (t u v) -> p t u v", t=2, u=8)
            for n, (i, j, coef) in enumerate(C222_NNZ[k]):
                sl = pp5[:, :, :, i, :, j]
                if n == 0:
                    eng.tensor_scalar(tv, sl, coef, None, op0=MULT)
                else:
                    eng.scalar_tensor_tensor(tv, sl, coef, tv,
                                             op0=MULT, op1=ADD)

        def dve_ak(k):
            tt_mul(ak_s[:, k * 8 * 256:(k + 1) * 8 * 256]
                   .rearrange("p (c z) -> p c z", c=8),
                   st_s.unsqueeze(1).to_broadcast([128, 8, 256]),
                   ball_s[:, k * 8 * 256:(k + 1) * 8 * 256]
                   .rearrange("p (c z) -> p c z", c=8))

        pp_s = big.tile([128, 3200], bf16, tag="pp")
        pp3 = pp_s.rearrange("p (t a b) -> p t a b", t=2, a=40)
        pp5 = pp_s.rearrange("p (t u i v j) -> p t u i v j", t=2, u=8, i=5,
                             v=8)
        tb3 = tb_s.rearrange("p (t a) -> p t a", t=2)
        tpk_s = [scr.tile([128, 128], bf16, name=f"tpk{k}", tag=f"tpk{k}")
                 for k in range(5)]

        def pool_ak(k):
            nc.gpsimd.tensor_tensor(
                ak_s[:, k * 8 * 256:(k + 1) * 8 * 256]
                .rearrange("p (c z) -> p c z", c=8),
                st_s.unsqueeze(1).to_broadcast([128, 8, 256]),
                ball_s[:, k * 8 * 256:(k + 1) * 8 * 256]
                .rearrange("p (c z) -> p c z", c=8), op=MULT)

        # Pool: t pair products, late akT slabs, tt2 diagonals
        nc.gpsimd.tensor_tensor(
            pp3, tb3.unsqueeze(3).to_broadcast([128, 2, 40, 40]),
            tb3.unsqueeze(2).to_broadcast([128, 2, 40, 40]), op=MULT)
        pool_ak(3)
        pool_ak(4)
        pool_ak(2)
        for d, ch, soff in W220_SLOTS:
            n = 20 if d == 4 else 40
            if d == 0:
                nc.gpsimd.tensor_tensor(tt2_s[ch][soff:soff + n, :],
                                        tt_s, tt_s, op=MULT)
            else:
                nc.gpsimd.tensor_tensor(tt2_s[ch][soff:soff + n, :],
                                        tt_s[0:n, :],
                                        p40[0:n, d * 256:(d + 1) * 256],
                                        op=MULT)

        # DVE: diagonal products, C222 chains, akT, then F rows
        tt_mul(s2p_s[:, 0:256], st_s, st_s)
        dve_s2p(0)
        tpk_chain(nc.vector, 0)
        dve_s2p(1)
        tpk_chain(nc.vector, 1)
        tpk_chain(nc.vector, 2)
        dve_ak(0)
        tpk_chain(nc.vector, 3)
        tpk_chain(nc.vector, 4)
        dve_ak(1)
        dve_s2p(2)
        dve_s2p(3)

        # ---- PE program: tpk transposes early, y0 groups with y2T/V-path
        # blocks injected to fill DMA-gated gaps ----
        tpkt_s = []
        y2t_s = [big.tile([128, 512], bf16, name=f"y2t_{k}", tag=f"y2t_{k}")
                 for k in range(5)]
        y2zm_s = [big.tile([128, 1280], bf16, name=f"y2zm_{tau}",
                           tag=f"y2zm_{tau}") for tau in range(2)]
        d_s = [big.tile([128, 1280], bf16, name=f"d_{tau}", tag=f"d_{tau}")
               for tau in range(2)]

        def pe_tpkt(ks):
            for k in ks:
                ptk_full = ptr.tile([128, 256], bf16, name=f"ptk{k}",
                                    tag="ptr")
                ptk = ptk_full[0:64, :]
                for tau in range(2):
                    nc.tensor.transpose(ptk[:, tau * 128:(tau + 1) * 128],
                                        tpk_s[k][:, tau * 64:(tau + 1) * 64],
                                        identb)
                tpkt = scr.tile([64, 256], bf16, name=f"tpkt{k}", tag="tpkt")
                tpkt_s.append(tpkt)
                nc.scalar.copy(out=tpkt, in_=ptk)

        def pe_y2mm(k):
            """y2T_k matmuls into psum + Act copies (no PE-side waits)."""
            y2full = py2.tile([128, 512], f32, name=f"y2full{k}", tag="py2")
            for wh in range(2):
                yps = y2full[:, wh * 256:(wh + 1) * 256]
                for c in range(8):
                    mm(yps,
                       wpr_s[:, c * 256 + wh * 128:c * 256 + wh * 128 + 128],
                       ak_s[:, (k * 8 + c) * 256:(k * 8 + c + 1) * 256],
                       start=(c == 0), stop=False)
                mm(yps, w222_s[:, wh * 128:(wh + 1) * 128], tpkt_s[k],
                   start=False, stop=True)
            nc.scalar.copy(out=y2t_s[k], in_=y2full)

        def pe_zmD(k):
            """z-major transposes of y2T_k + D_k matmuls (reads SBUF y2t)."""
            for tau in range(2):
                ptz = ptr.tile([128, 256], bf16, name=f"ptz{k}{tau}",
                               tag="ptr")
                for wh in range(2):
                    nc.tensor.transpose(
                        ptz[:, wh * 128:(wh + 1) * 128],
                        y2t_s[k][:, wh * 256 + tau * 128:
                                 wh * 256 + tau * 128 + 128],
                        identb)
                nc.scalar.copy(out=y2zm_s[tau][:, k * 256:(k + 1) * 256],
                               in_=ptz)
            for tau in range(2):
                dps = p256.tile([128, 256], f32, name=f"dps{k}{tau}",
                                tag="p256")
                for uh in range(2):
                    mm(dps,
                       y2t_s[k][:, uh * 256 + tau * 128:
                                uh * 256 + tau * 128 + 128],
                       v222_s[:, uh * 256:(uh + 1) * 256],
                       start=(uh == 0), stop=(uh == 1))
                nc.scalar.copy(out=d_s[tau][:, k * 256:(k + 1) * 256],
                               in_=dps)

        pe_y0(0)
        pe_y0(1)
        pe_tpkt([0])
        pe_y2mm(0)
        pe_y0(2)
        pe_tpkt([1])
        pe_zmD(0)
        pe_y2mm(1)
        pe_tpkt([2])
        pe_y2mm(2)
        pe_zmD(1)
        pe_y0(3)
        pe_tpkt([3])
        pe_y2mm(3)
        pe_zmD(2)
        pe_tpkt([4])
        pe_y2mm(4)
        pe_zmD(3)
        # W220 contribution into y0 PSUM (before the final y0 group, whose
        # last chunk carries the stop flag)
        for ch in range(3):
            for wh in range(2):
                mm(y0ps[wh], w220_s[ch][:, wh * 128:(wh + 1) * 128],
                   tt2_s[ch], start=False, stop=False)
        pe_zmD(4)
        pe_y0(4)
        pe_y0(5)
        y0t_s = big.tile([128, 512], bf16, tag="y0t")
        for wh in range(2):
            nc.scalar.copy(out=y0t_s[:, wh * 256:(wh + 1) * 256],
                           in_=y0ps[wh])

        # af (z-major) from y0T
        af_s = [big.tile([128, 256], bf16, name=f"af_{tau}",
                         tag=f"af_{tau}") for tau in range(2)]
        for tau in range(2):
            aps = p256.tile([128, 256], f32, name=f"aps{tau}", tag="p256")
            for uh in range(2):
                mm(aps,
                   y0t_s[:, uh * 256 + tau * 128:uh * 256 + tau * 128 + 128],
                   v02f_s[:, uh * 256:(uh + 1) * 256],
                   start=(uh == 0), stop=(uh == 1))
            nc.scalar.copy(out=af_s[tau], in_=aps)

        # ---- F rows + o01 rows via fused accum products (all DVE; the Pool
        # engine has no TensorScalarPtr on real hardware) ----
        fmat_s = [scr.tile([128, 20], bf16, name=f"fmat{tau}", tag="fmat")
                  for tau in range(2)]

        def f_row(tau, r, i, j):
            sink = scr.tile([128, 256], bf16, name="sinkD", tag="sinkD")
            stt(nc.vector, sink, d_s[tau][:, i * 256:(i + 1) * 256],
                y2zm_s[tau][:, j * 256:(j + 1) * 256],
                accum_out=fmat_s[tau][:, r:r + 1])

        def o_row(tau, k):
            sink = scr.tile([128, 256], bf16, name="sinkD", tag="sinkD")
            stt(nc.vector, sink, af_s[tau],
                y2zm_s[tau][:, k * 256:(k + 1) * 256],
                accum_out=fmat_s[tau][:, 15 + k:16 + k])

        for m in range(5):
            for r, (i, j) in enumerate(F_PAIRS):
                if max(i, j) == m:
                    for tau in range(2):
                        f_row(tau, r, i, j)

        # ---- final combine: F-part (rows 0:15) runs while the o rows are
        # still accumulating; o-part joins the same PSUM group after ----
        outt = scr.tile([128, 16], f32, tag="outt")
        ops_t = []
        for tau in range(2):
            ptf_full = ptr.tile([128, 256], bf16, name=f"ptf{tau}",
                                tag="ptr")
            ptf = ptf_full[0:15, 0:128]
            nc.tensor.transpose(ptf, fmat_s[tau][:, 0:15], identb)
            fmT = scr.tile([20, 128], bf16, name=f"fmT{tau}", tag="fmT")
            nc.scalar.copy(out=fmT[0:15], in_=ptf)
            ops = p256.tile([128, 8], f32, name=f"ops{tau}", tag="p256")
            ops_t.append(ops)
            mm(ops, fmT[0:15], comb_s[0:15], start=True, stop=False)
        for m in range(5):
            for tau in range(2):
                o_row(tau, m)
        for tau in range(2):
            ptf_full = ptr.tile([128, 256], bf16, name=f"ptfo{tau}",
                                tag="ptr")
            ptfo = ptf_full[0:5, 0:128]
            nc.tensor.transpose(ptfo, fmat_s[tau][:, 15:20], identb)
            fmTo = scr.tile([5, 128], bf16, name=f"fmTo{tau}", tag="fmTo")
            nc.scalar.copy(out=fmTo, in_=ptfo)
            mm(ops_t[tau], fmTo, combO_s, start=False, stop=True)
            nc.vector.tensor_copy(outt[:, tau * 8:(tau + 1) * 8],
                                  ops_t[tau])
        dma(out=out_d.rearrange("(t p) w -> p t w", p=128),
            in_=outt.rearrange("p (t w) -> p t w", t=2))

    nc.compile()
    return nc


_CACHE = {}


def _get_nc():
    if "nc" not in _CACHE:
        _CACHE["nc"] = build_nc()
    return _CACHE["nc"]


def make_in_maps(inputs):
    shared, percore = host_prep(**{k: np.asarray(v, dtype=np.float32)
                                   for k, v in inputs.items()})
    return [dict(shared, **pc) for pc in percore]


def kernel(**inputs):
    from concourse.bass_utils import run_bass_kernel_spmd

    in_maps = make_in_maps(inputs)
    nc = _get_nc()
    res = run_bass_kernel_spmd(nc, in_maps, list(range(N_CORES)))
    out = np.concatenate([res.results[c]["out"][:, :5]
                          for c in range(N_CORES)], 0)
    return np.ascontiguousarray(out.astype(np.float32))

